# revision 6
# baseline (speedup 1.0000x reference)
"""Bidirectional GRU encoder (packed-sequence semantics) on 8 TRN2 NeuronCores.

Sharding: direction x batch-quarter.  Cores 0-3 run the left-to-right GRU on
batch quarters, cores 4-7 run the right-to-left GRU (on host-reversed token
streams) on batch quarters.  Each core holds 16 of the 64 sequences.

Device kernel (per core, identical SPMD program, different inputs):
  - input-projection GEMMs (x @ W{r,z,h}.T + b) computed chunk-by-chunk
  - the 2048-step GRU recurrence with U-stationary [H-partition, B-free]
    layout; pre-activations re-injected into PSUM via an identity matmul,
    recurrent matmuls accumulate on top; sigmoid/tanh on ACT; elementwise on
    DVE writing the hidden state directly into the output ring buffer.
  - all matmul operands bf16 (fp32 PSUM accumulate); hidden state bf16.

Host: embedding gather (pure data movement), sequence reversal indices, final
masking / flip-back / dtype assembly.
"""

import os
import sys

for _p in ("/opt/trn_rl_repo", "/root/.axon_site/_ro/trn_rl_repo"):
    if os.path.isdir(_p) and _p not in sys.path:
        sys.path.append(_p)

import numpy as np
import ml_dtypes

BF16 = ml_dtypes.bfloat16

L, B, H, E = 2048, 64, 256, 256
NCORES = 8
BL = 16          # sequences per core (dir-sharded: 4 cores per direction)
TCH = 64         # recurrence steps per chunk (2 chunks per For_i body)

_PROGRAM_CACHE = {}


def _build_program(steps=L, tch=TCH):
    import concourse.bacc as bacc
    import concourse.tile as tile
    import concourse.bass as bass
    import concourse.mybir as mybir

    dt = mybir.dt
    AF = mybir.ActivationFunctionType
    OP = mybir.AluOpType

    nc = bacc.Bacc(
        "TRN2",
        target_bir_lowering=False,
        debug=False,
        num_devices=NCORES,
    )

    # ---- DRAM I/O ----------------------------------------------------------
    xT = nc.dram_tensor("xT", [2, 128, steps, BL], dt.bfloat16, kind="ExternalInput").ap()
    U_lhsT = nc.dram_tensor("U_lhsT", [2, 128, 768], dt.bfloat16, kind="ExternalInput").ap()
    W_lhsT = nc.dram_tensor("W_lhsT", [2, 128, 768], dt.bfloat16, kind="ExternalInput").ap()
    biasT = nc.dram_tensor("biasT", [128, 6], dt.float32, kind="ExternalInput").ap()
    ident = nc.dram_tensor("ident", [128, 128], dt.bfloat16, kind="ExternalInput").ap()
    out_dev = nc.dram_tensor("out_dev", [128, 2, steps, BL], dt.bfloat16, kind="ExternalOutput").ap()

    with tile.TileContext(nc) as tc:
        import contextlib
        ctx = contextlib.ExitStack()
        with ctx:
            const = ctx.enter_context(tc.tile_pool(name="const", bufs=1))
            state = ctx.enter_context(tc.tile_pool(name="state", bufs=1))
            xpool = ctx.enter_context(tc.tile_pool(name="xpool", bufs=2))
            prepool = ctx.enter_context(tc.tile_pool(name="prepool", bufs=2))
            spool = ctx.enter_context(tc.tile_pool(name="spool", bufs=3))
            gpsum = ctx.enter_context(tc.tile_pool(name="gpsum", bufs=3, space="PSUM"))
            spsum = ctx.enter_context(tc.tile_pool(name="spsum", bufs=2, space="PSUM"))

            # ---- constants in SBUF ----------------------------------------
            U_sb = const.tile([128, 2, 768], dt.bfloat16)
            W_sb = const.tile([128, 2, 768], dt.bfloat16)
            for k in (0, 1):
                nc.sync.dma_start(U_sb[:, k, :], U_lhsT[k])
                nc.sync.dma_start(W_sb[:, k, :], W_lhsT[k])
            bias_sb = const.tile([128, 6], dt.float32)
            nc.sync.dma_start(bias_sb[:], biasT[:])
            I_sb = const.tile([128, 128], dt.bfloat16)
            nc.sync.dma_start(I_sb[:], ident[:])

            # ---- persistent state -----------------------------------------
            hcarry = state.tile([128, 2, BL], dt.bfloat16)
            nc.gpsimd.memset(hcarry[:], 0.0)
            obufs = [state.tile([128, 2, tch, BL], dt.bfloat16,
                                name=f"obuf{i}", tag=f"obuf{i}")
                     for i in (0, 1)]

            def gru_chunk(c_off, obuf, h_entry):
                """Input GEMM + tch recurrence steps for one chunk.

                c_off: scalar (static or loop-var expr) element offset in steps
                obuf:  [128, 2, tch, BL] output/state ring for this chunk
                h_entry: [128, 2, BL] hidden state entering the chunk
                """
                # x chunk DMA (both halves of E on partitions)
                xk = []
                for k in (0, 1):
                    t_ = xpool.tile([128, tch, BL], dt.bfloat16, tag=f"x{k}")
                    nc.sync.dma_start(t_[:], xT[k, :, bass.ds(c_off, tch), :])
                    xk.append(t_)

                # input projections: pre[m, t, b] = sum_k W.T x  (+ bias)
                pre = prepool.tile([128, 6, tch, BL], dt.bfloat16, tag="pre")
                nh = max(1, (tch * BL) // 512)   # N-halves of 512 cols
                tsub = tch // nh
                for m in range(6):
                    for hh in range(nh):
                        ps = gpsum.tile([128, tsub * BL], dt.float32, tag="gemm")
                        nc.tensor.matmul(
                            ps[:], W_sb[:, 0, m * 128:(m + 1) * 128],
                            xk[0][:, hh * tsub:(hh + 1) * tsub, :],
                            start=True, stop=False, skip_group_check=True)
                        nc.tensor.matmul(
                            ps[:], W_sb[:, 1, m * 128:(m + 1) * 128],
                            xk[1][:, hh * tsub:(hh + 1) * tsub, :],
                            start=False, stop=True, skip_group_check=True)
                        nc.scalar.activation(
                            pre[:, m, hh * tsub:(hh + 1) * tsub, :], ps[:],
                            AF.Identity, bias=bias_sb[:, m:m + 1])

                # recurrence
                for t in range(tch):
                    hprev = h_entry if t == 0 else obuf[:, :, t - 1, :]
                    ps = spsum.tile([128, 6, BL], dt.float32, tag="step")
                    # pre-activation injection (identity matmul, N = 6*BL)
                    nc.tensor.matmul(ps[:, :, :], I_sb[:], pre[:, :, t, :],
                                     start=True, stop=False, skip_group_check=True)
                    # r/z recurrent matmuls accumulate on top
                    for m in range(4):
                        for k in (0, 1):
                            nc.tensor.matmul(
                                ps[:, m, :], U_sb[:, k, m * 128:(m + 1) * 128],
                                hprev[:, k, :],
                                start=False, stop=(k == 1), skip_group_check=True)
                    rz = spool.tile([128, 4, BL], dt.bfloat16, tag="rz")
                    nc.scalar.activation(rz[:], ps[:, 0:4, :], AF.Sigmoid)
                    rh = spool.tile([128, 2, BL], dt.bfloat16, tag="rh")
                    nc.vector.tensor_mul(rh[:], rz[:, 0:2, :], hprev)
                    # w = (z - 1) * h   (off critical path)
                    w_ = spool.tile([128, 2, BL], dt.bfloat16, tag="w")
                    nc.vector.scalar_tensor_tensor(
                        w_[:], rz[:, 2:4, :], 1.0, hprev, OP.subtract, OP.mult)
                    # candidate matmuls
                    for m in (4, 5):
                        for k in (0, 1):
                            nc.tensor.matmul(
                                ps[:, m, :], U_sb[:, k, m * 128:(m + 1) * 128],
                                rh[:, k, :],
                                start=False, stop=(k == 1), skip_group_check=True)
                    hp = spool.tile([128, 2, BL], dt.bfloat16, tag="hp")
                    nc.scalar.activation(hp[:], ps[:, 4:6, :], AF.Tanh)
                    u_ = spool.tile([128, 2, BL], dt.bfloat16, tag="u")
                    nc.vector.tensor_mul(u_[:], rz[:, 2:4, :], hp[:])
                    # h' = z*hp - (z-1)*h  -> straight into the output ring
                    nc.vector.tensor_sub(obuf[:, :, t, :], u_[:], w_[:])

                nc.sync.dma_start(out_dev[:, :, bass.ds(c_off, tch), :], obuf[:])

            nsteps_pair = 2 * tch
            assert steps % nsteps_pair == 0
            import concourse.mybir as _mybir
            with tc.For_i(0, steps, nsteps_pair,
                          hint_engines=(_mybir.EngineType.PE,)) as it:
                gru_chunk(it, obufs[0], hcarry[:])
                gru_chunk(it + tch, obufs[1], obufs[0][:, :, tch - 1, :])
                nc.vector.tensor_copy(hcarry[:], obufs[1][:, :, tch - 1, :])

    nc.compile()
    return nc


def _get_program(steps=L, tch=TCH):
    key = (steps, tch)
    if key not in _PROGRAM_CACHE:
        _PROGRAM_CACHE[key] = _build_program(steps, tch)
    return _PROGRAM_CACHE[key]


def _host_inputs(tokens, lengths, emb, weights):
    """Build the 8 per-core input maps. weights: dict with ltr_*/rtl_* arrays."""
    ident = np.eye(128, dtype=np.float32).astype(BF16)
    t_idx = np.arange(L, dtype=np.int64)[:, None]
    in_maps = []
    dirmats = {}
    for d, pfx in ((0, "ltr"), (1, "rtl")):
        U_all = np.concatenate(
            [weights[f"{pfx}_Ur"], weights[f"{pfx}_Uz"], weights[f"{pfx}_Uh"]], axis=0)
        W_all = np.concatenate(
            [weights[f"{pfx}_Wr"], weights[f"{pfx}_Wz"], weights[f"{pfx}_Wh"]], axis=0)
        b_all = np.concatenate(
            [weights[f"{pfx}_br"], weights[f"{pfx}_bz"], weights[f"{pfx}_bh"]], axis=0)
        dirmats[d] = (
            np.ascontiguousarray(U_all.T.reshape(2, 128, 768)).astype(BF16),
            np.ascontiguousarray(W_all.T.reshape(2, 128, 768)).astype(BF16),
            np.ascontiguousarray(b_all.reshape(6, 128).T).astype(np.float32),
        )
    for c in range(NCORES):
        d = c // 4
        q = c % 4
        bsl = slice(BL * q, BL * (q + 1))
        tok = tokens[:, bsl]
        if d == 1:
            ridx = lengths[None, bsl].astype(np.int64) - 1 - t_idx
            cidx = np.clip(ridx, 0, L - 1)
            tok = np.take_along_axis(tok, cidx, axis=0)
        x = emb[tok]                                   # [L, BL, E] f32
        xT_ = np.ascontiguousarray(x.transpose(2, 0, 1)).reshape(2, 128, L, BL)
        U_, W_, b_ = dirmats[d]
        in_maps.append({
            "xT": xT_.astype(BF16),
            "U_lhsT": U_,
            "W_lhsT": W_,
            "biasT": b_,
            "ident": ident,
        })
    return in_maps


def _assemble(results, lengths):
    """results: list of 8 dicts with 'out_dev' [128, 2, L, BL] bf16."""
    t_idx = np.arange(L, dtype=np.int64)[:, None]
    mask = (t_idx < lengths[None, :].astype(np.int64))          # [L, B]

    def halves(cores):
        hs = []
        for c in cores:
            a = np.asarray(results[c]["out_dev"]).astype(np.float32)
            # [p, hc, t, b] -> [t, b, hc, p] -> [t, b, 256]
            hs.append(a.transpose(2, 3, 1, 0).reshape(L, BL, H))
        return np.concatenate(hs, axis=1)                       # [L, B, H]

    ltr_h = halves(range(4))
    rev_h = halves(range(4, 8))
    out_ltr = np.where(mask[:, :, None], ltr_h, 0.0)
    ridx = lengths[None, :].astype(np.int64) - 1 - t_idx
    cidx = np.clip(ridx, 0, L - 1)
    flipped = np.take_along_axis(rev_h, cidx[:, :, None], axis=0)
    out_rtl = np.where(mask[:, :, None], flipped, 0.0)
    return np.concatenate([out_ltr, out_rtl], axis=-1).astype(np.float32)


LAST_PROFILE = None


def _install_ntff_shim():
    """The agent image's `antenv` lacks `axon_hooks`; synthesize it and
    register the ctypes NTFF hook so run_bass_kernel_spmd(trace=True) works."""
    import types
    if "antenv.axon_hooks" not in sys.modules:
        mod = types.ModuleType("antenv.axon_hooks")
        mod._hook = None

        def set_axon_ntff_profile_hook(h):
            mod._hook = h

        def get_axon_ntff_profile_hook():
            return mod._hook

        mod.set_axon_ntff_profile_hook = set_axon_ntff_profile_hook
        mod.get_axon_ntff_profile_hook = get_axon_ntff_profile_hook
        sys.modules["antenv.axon_hooks"] = mod
        import antenv
        antenv.axon_hooks = mod
    mod = sys.modules["antenv.axon_hooks"]
    if mod._hook is None:
        from trn_agent_boot.trn_boot import _ntff_profile_via_ctypes
        hook = _ntff_profile_via_ctypes("/opt/axon/libaxon_pjrt.so")
        if hook is None:
            raise RuntimeError("libaxon_pjrt.so lacks profile symbols")
        mod._hook = hook
    # artifact upload needs a bucket this container doesn't have
    import concourse.bass_utils as bu
    bu.upload_artifacts = lambda d: d


def kernel(_profile=False, **inputs):
    global LAST_PROFILE
    from concourse.bass_utils import run_bass_kernel_spmd

    tokens = np.asarray(inputs["tokens"])
    lengths = np.asarray(inputs["lengths"])
    emb = np.asarray(inputs["emb"], dtype=np.float32)

    nc = _get_program()
    in_maps = _host_inputs(tokens, lengths, emb, inputs)
    import tempfile
    kw = {}
    if _profile:
        try:
            _install_ntff_shim()
            kw = dict(trace=True, tmpdir=tempfile.mkdtemp(prefix="gru_trace_"))
        except Exception as e:
            print(f"profiling unavailable ({e}); running untraced", file=sys.stderr)
    res = run_bass_kernel_spmd(nc, in_maps, list(range(NCORES)), **kw)
    if _profile:
        LAST_PROFILE = {
            "exec_time_ns": res.exec_time_ns,
            "trace_dir": kw.get("tmpdir"),
        }
    return _assemble(res.results, lengths)
